# revision 45
# baseline (speedup 1.0000x reference)
"""Bass/Trainium2 kernel for masked attention returning (x, p_attn).

Problem: B=2, H=16, S=2048, DK=64.
  scores = q @ k^T / sqrt(DK); masked fill -1e9 where mask==0;
  p = softmax(scores); x = p @ v; return (x, p).

Sharding: B*H = 32 heads, data-parallel over 8 NeuronCores (4 heads each).
Cores 0-3 handle batch 0, cores 4-7 batch 1 (mask is per-batch).

Per-core algorithm (all layouts chosen to avoid on-device input transposes;
q^T / k^T / bf16 casts are prepared on host):
  s[q,k] = qT^T @ kT           (PE, PSUM f32, q pre-scaled by 1/8)
  u = exp(s)                   (ACT, PSUM->SBUF bf16)
  pu = u * mask                (DVE tensor_mul bf16; row sums come from PV)
  pu^T via PE transpose + PSUM->SBUF copies  (feeds PV matmul)
  xT[d,q] += v65^T @ pu^T chunks  (PE; v65 has a ones column -> row 64 = r)
  recip = 1/r                  (DVE reciprocal)
  x = transpose(xT) * recip    (PE transpose + DVE tensor_scalar)
  p = pu * recip -> f32        (DVE tensor_scalar, DMA out)
"""

import numpy as np
import ml_dtypes

B, H, S, DK = 2, 16, 2048, 64
N_CORES = 8
HPC = (B * H) // N_CORES  # heads per core = 4
SCALE = 1.0 / np.sqrt(DK)

QB = 128          # q rows per block (partition dim)
KSB = 512         # k cols per matmul (PSUM bank)
EXPW = 1024       # exp/TTR tile width (2 PSUM banks)
NQB = S // QB     # 16 q-blocks
NSUP = 4          # q-supers (4 q-blocks each)
NKC = S // QB     # 16 k-chunks of 128

_nc_cache = None


def _build_nc():
    import concourse.tile as tile
    from concourse import bacc, mybir
    from concourse.masks import make_identity

    f32 = mybir.dt.float32
    bf16 = mybir.dt.bfloat16
    Alu = mybir.AluOpType
    Act = mybir.ActivationFunctionType

    nc = bacc.Bacc("TRN2", target_bir_lowering=False, debug=False,
                   num_devices=N_CORES)

    qT_d = nc.declare_dram_parameter("qT", [HPC, DK, S], bf16, isOutput=False)
    kT_d = nc.declare_dram_parameter("kT", [HPC, DK, S], bf16, isOutput=False)
    v_d = nc.declare_dram_parameter("v", [HPC, S, DK + 1], bf16, isOutput=False)
    m_d = nc.declare_dram_parameter("maskb", [S, S], bf16, isOutput=False)
    p_d = nc.declare_dram_parameter("p_out", [HPC, S, S], f32, isOutput=True)
    x_d = nc.declare_dram_parameter("x_out", [HPC, S, DK], f32, isOutput=True)

    with tile.TileContext(nc) as tc:
        import contextlib
        ctx = contextlib.ExitStack()
        with ctx:
            consts = ctx.enter_context(tc.tile_pool(name="consts", bufs=1))
            maskp = ctx.enter_context(tc.tile_pool(name="maskp", bufs=1))
            qkv = ctx.enter_context(tc.tile_pool(name="qkv", bufs=3))
            pu_p = ctx.enter_context(tc.tile_pool(name="pu", bufs=8))
            u_p = ctx.enter_context(tc.tile_pool(name="u", bufs=3))
            pt_p = ctx.enter_context(tc.tile_pool(name="pt", bufs=28))
            pf_p = ctx.enter_context(tc.tile_pool(name="pf", bufs=4))
            small = ctx.enter_context(tc.tile_pool(name="small", bufs=8))
            xo_p = ctx.enter_context(tc.tile_pool(name="xo", bufs=4))
            s_ps = ctx.enter_context(tc.tile_pool(name="sps", bufs=2, space="PSUM"))
            tp_ps = ctx.enter_context(tc.tile_pool(name="tpps", bufs=4, space="PSUM"))

            id_bf = consts.tile([128, 128], bf16)
            make_identity(nc, id_bf)
            id_f32 = consts.tile([128, 128], f32)
            make_identity(nc, id_f32)

            # head-0 inputs first (unblocks the first super quickly)
            qkv_tiles = {}

            def load_head(h):
                if h in qkv_tiles:
                    return qkv_tiles[h]
                qT_sb = qkv.tile([DK, S], bf16, tag="qT", name="qT_sb")
                nc.sync.dma_start(out=qT_sb[:], in_=qT_d[h])
                kT_sb = qkv.tile([DK, S], bf16, tag="kT", name="kT_sb")
                nc.sync.dma_start(out=kT_sb[:], in_=kT_d[h])
                v_sb = qkv.tile([128, NKC, DK + 1], bf16, tag="v", name="v_sb")
                nc.sync.dma_start(
                    out=v_sb[:], in_=v_d[h].rearrange("(c p) d -> p c d", p=128)
                )
                qkv_tiles[h] = (qT_sb, kT_sb, v_sb)
                return qkv_tiles[h]

            load_head(0)

            # resident mask tiles, one per q-block
            mts = []
            for qb in range(NQB):
                mt = maskp.tile([QB, S], bf16, tag=f"m{qb}", name=f"mask{qb}")
                nc.sync.dma_start(out=mt[:], in_=m_d[qb * QB:(qb + 1) * QB, :])
                mts.append(mt)

            def get_mask(qb):
                return mts[qb]

            def emit_B_chunk(st, i4):
                for c16 in range(i4 * 4, i4 * 4 + 4):
                    off = c16 * 128
                    tpt = tp_ps.tile([128, KSB], f32, tag="tpf",
                                     name="tpt", bufs=4)
                    for q4 in range(4):
                        nc.tensor.matmul(
                            tpt[:, q4 * 128:(q4 + 1) * 128],
                            st["pu"][q4][:, off:off + 128],
                            id_bf[:],
                            start=True, stop=True,
                        )
                    pt_t = pt_p.tile([128, KSB], bf16, tag="pT")
                    if c16 % 8 < 5:
                        nc.scalar.copy(pt_t[:], tpt[:])
                    else:
                        nc.vector.tensor_copy(pt_t[:], tpt[:])
                    st["pT"].append(pt_t)

            def emit_B_tail(st):
                h, qs, last = st["h"], st["qs"], st["last"]
                pu, pT, v_sb = st["pu"], st["pT"], st["v_sb"]
                xT = tp_ps.tile([128, KSB], f32, tag="tpf", bufs=4)
                for kc in range(NKC):
                    nc.tensor.matmul(
                        xT[0:DK + 1, :],
                        v_sb[:, kc, :],
                        pT[kc][:],
                        start=(kc == 0), stop=(kc == NKC - 1),
                    )
                xT_sb = small.tile([DK + 1, KSB], f32, tag="xT", bufs=2)
                nc.scalar.copy(xT_sb[:], xT[0:DK + 1, :])

                xn = tp_ps.tile([128, KSB], f32, tag="tpf", bufs=4)
                for q4 in range(4):
                    nc.tensor.transpose(
                        xn[:, q4 * (DK + 1):(q4 + 1) * (DK + 1)],
                        xT_sb[:, q4 * 128:(q4 + 1) * 128],
                        id_f32[0:DK + 1, 0:DK + 1],
                    )
                for q4 in range(4):
                    qb = qs * 4 + q4
                    rc = small.tile([QB, 1], f32, tag="rc")
                    nc.vector.reciprocal(rc[:], xn[:, q4 * (DK + 1) + DK:
                                                   q4 * (DK + 1) + DK + 1])
                    x_sb = xo_p.tile([QB, DK], f32, tag="x")
                    nc.vector.tensor_scalar(
                        out=x_sb[:],
                        in0=xn[:, q4 * (DK + 1):q4 * (DK + 1) + DK],
                        scalar1=rc[:], scalar2=None, op0=Alu.mult,
                    )
                    nc.sync.dma_start(
                        out=x_d[h, qb * QB:(qb + 1) * QB, :], in_=x_sb[:]
                    )
                    if not last:
                        pf = pf_p.tile([QB, S], f32, tag="pf")
                        nc.vector.tensor_scalar(
                            out=pf[:], in0=pu[q4][:],
                            scalar1=rc[:], scalar2=None, op0=Alu.mult,
                        )
                        nc.sync.dma_start(
                            out=p_d[h, qb * QB:(qb + 1) * QB, :],
                            in_=pf[:]
                        )

            for h in range(HPC):
                qT_sb, kT_sb, v_sb = load_head(h)
                if h + 1 < HPC:
                    load_head(h + 1)  # prefetch next head's inputs

                for qs in range(NSUP):
                    last = (h == HPC - 1 and qs == NSUP - 1)
                    pu = [pu_p.tile([QB, S], bf16, tag="pu", name="pu") for _ in range(4)]
                    st = {"h": h, "qs": qs, "last": last, "pu": pu,
                          "pT": [], "v_sb": v_sb}

                    for q4 in range(4):
                        qb = qs * 4 + q4
                        u_t = u_p.tile([QB, S], bf16, tag="u", name="u_t")
                        for k2 in range(S // EXPW):
                            s_t = s_ps.tile([QB, EXPW], f32, tag="s",
                                            name="s_t")
                            for j in range(EXPW // KSB):
                                nc.tensor.matmul(
                                    s_t[:, j * KSB:(j + 1) * KSB],
                                    qT_sb[:, qb * QB:(qb + 1) * QB],
                                    kT_sb[:, k2 * EXPW + j * KSB:
                                          k2 * EXPW + (j + 1) * KSB],
                                    start=True, stop=True,
                                )
                            nc.scalar.activation(
                                u_t[:, k2 * EXPW:(k2 + 1) * EXPW], s_t[:],
                                Act.Exp)
                        if not last:
                            nc.vector.tensor_mul(pu[q4][:], u_t[:],
                                                 get_mask(qb)[:])
                        else:
                            # final super: fused mask-mul + row-sum so the
                            # p normalize/DMA overlaps the PV chain instead
                            # of trailing it (shorter kernel tail)
                            r_e = small.tile([QB, 1], f32, tag="re",
                                             name="r_e")
                            nc.vector.scalar_tensor_tensor(
                                out=pu[q4][:], in0=u_t[:], scalar=1.0,
                                in1=get_mask(qb)[:], op0=Alu.mult,
                                op1=Alu.mult, accum_out=r_e[:])
                            rc_e = small.tile([QB, 1], f32, tag="rce",
                                              name="rc_e")
                            nc.vector.reciprocal(rc_e[:], r_e[:])
                            pf_e = pf_p.tile([QB, S], f32, tag="pf",
                                             name="pf_e")
                            nc.vector.tensor_scalar(
                                out=pf_e[:], in0=pu[q4][:], scalar1=rc_e[:],
                                scalar2=None, op0=Alu.mult)
                            nc.sync.dma_start(
                                out=p_d[h, qb * QB:(qb + 1) * QB, :],
                                in_=pf_e[:])
                    for i4 in range(4):
                        emit_B_chunk(st, i4)
                    emit_B_tail(st)

    nc.compile()
    return nc


def _get_nc():
    global _nc_cache
    if _nc_cache is None:
        _nc_cache = _build_nc()
    return _nc_cache


def _prep_in_maps(query, key, value, mask):
    bf = ml_dtypes.bfloat16
    q = (query.reshape(B * H, S, DK).transpose(0, 2, 1) * SCALE).astype(bf)
    kt = key.reshape(B * H, S, DK).transpose(0, 2, 1).astype(bf)
    v = np.ones((B * H, S, DK + 1), dtype=bf)
    v[:, :, :DK] = value.reshape(B * H, S, DK).astype(bf)
    mb = mask.reshape(B, S, S).astype(bf)
    in_maps = []
    for c in range(N_CORES):
        h0 = c * HPC
        in_maps.append({
            "qT": np.ascontiguousarray(q[h0:h0 + HPC]),
            "kT": np.ascontiguousarray(kt[h0:h0 + HPC]),
            "v": np.ascontiguousarray(v[h0:h0 + HPC]),
            "maskb": np.ascontiguousarray(mb[h0 // H]),
        })
    return in_maps


def _gather(results):
    p = np.concatenate([results[c]["p_out"] for c in range(N_CORES)], axis=0)
    x = np.concatenate([results[c]["x_out"] for c in range(N_CORES)], axis=0)
    return (x.reshape(B, H, S, DK), p.reshape(B, H, S, S))


def run(query, key, value, mask, trace=False):
    from concourse.bass_utils import run_bass_kernel_spmd

    nc = _get_nc()
    in_maps = _prep_in_maps(query, key, value, mask)
    res = None
    last_err = None
    for attempt in range(3):
        try:
            res = run_bass_kernel_spmd(
                nc, in_maps, core_ids=list(range(N_CORES)), trace=trace
            )
            break
        except Exception as e:  # transient device wedge: retry
            last_err = e
            import time
            time.sleep(5)
    if res is None:
        raise last_err
    x, p = _gather(res.results)
    return x, p, res


def kernel(query, key, value, mask):
    x, p, _ = run(query, key, value, mask)
    return (x, p)


# revision 46
# speedup vs baseline: 1.0759x; 1.0759x over previous
"""Bass/Trainium2 kernel for masked attention returning (x, p_attn).

Problem: B=2, H=16, S=2048, DK=64.
  scores = q @ k^T / sqrt(DK); masked fill -1e9 where mask==0;
  p = softmax(scores); x = p @ v; return (x, p).

Sharding: B*H = 32 heads, data-parallel over 8 NeuronCores (4 heads each).
Cores 0-3 handle batch 0, cores 4-7 batch 1 (mask is per-batch).

Per-core algorithm (all layouts chosen to avoid on-device input transposes;
q^T / k^T / bf16 casts are prepared on host):
  s[q,k] = qT^T @ kT           (PE, PSUM f32, q pre-scaled by 1/8)
  u = exp(s)                   (ACT, PSUM->SBUF bf16)
  pu = u * mask                (DVE tensor_mul bf16; row sums come from PV)
  pu^T via PE transpose + PSUM->SBUF copies  (feeds PV matmul)
  xT[d,q] += v65^T @ pu^T chunks  (PE; v65 has a ones column -> row 64 = r)
  recip = 1/r                  (DVE reciprocal)
  x = transpose(xT) * recip    (PE transpose + DVE tensor_scalar)
  p = pu * recip -> f32        (DVE tensor_scalar, DMA out)
"""

import numpy as np
import ml_dtypes

B, H, S, DK = 2, 16, 2048, 64
N_CORES = 8
HPC = (B * H) // N_CORES  # heads per core = 4
SCALE = 1.0 / np.sqrt(DK)

QB = 128          # q rows per block (partition dim)
KSB = 512         # k cols per matmul (PSUM bank)
EXPW = 1024       # exp/TTR tile width (2 PSUM banks)
NQB = S // QB     # 16 q-blocks
NSUP = 4          # q-supers (4 q-blocks each)
NKC = S // QB     # 16 k-chunks of 128

_nc_cache = None


def _build_nc():
    import concourse.tile as tile
    from concourse import bacc, mybir
    from concourse.masks import make_identity

    f32 = mybir.dt.float32
    bf16 = mybir.dt.bfloat16
    Alu = mybir.AluOpType
    Act = mybir.ActivationFunctionType

    nc = bacc.Bacc("TRN2", target_bir_lowering=False, debug=False,
                   num_devices=N_CORES)

    qT_d = nc.declare_dram_parameter("qT", [HPC, DK, S], bf16, isOutput=False)
    kT_d = nc.declare_dram_parameter("kT", [HPC, DK, S], bf16, isOutput=False)
    v_d = nc.declare_dram_parameter("v", [HPC, S, DK + 1], bf16, isOutput=False)
    m_d = nc.declare_dram_parameter("maskb", [S, S], bf16, isOutput=False)
    p_d = nc.declare_dram_parameter("p_out", [HPC, S, S], f32, isOutput=True)
    x_d = nc.declare_dram_parameter("x_out", [HPC, S, DK], f32, isOutput=True)

    with tile.TileContext(nc) as tc:
        import contextlib
        ctx = contextlib.ExitStack()
        with ctx:
            consts = ctx.enter_context(tc.tile_pool(name="consts", bufs=1))
            maskp = ctx.enter_context(tc.tile_pool(name="maskp", bufs=1))
            qkv = ctx.enter_context(tc.tile_pool(name="qkv", bufs=3))
            pu_p = ctx.enter_context(tc.tile_pool(name="pu", bufs=8))
            u_p = ctx.enter_context(tc.tile_pool(name="u", bufs=3))
            pt_p = ctx.enter_context(tc.tile_pool(name="pt", bufs=28))
            pf_p = ctx.enter_context(tc.tile_pool(name="pf", bufs=4))
            small = ctx.enter_context(tc.tile_pool(name="small", bufs=8))
            xo_p = ctx.enter_context(tc.tile_pool(name="xo", bufs=4))
            s_ps = ctx.enter_context(tc.tile_pool(name="sps", bufs=2, space="PSUM"))
            tp_ps = ctx.enter_context(tc.tile_pool(name="tpps", bufs=4, space="PSUM"))

            id_bf = consts.tile([128, 128], bf16)
            make_identity(nc, id_bf)
            id_f32 = consts.tile([128, 128], f32)
            make_identity(nc, id_f32)

            # head-0 inputs first (unblocks the first super quickly)
            qkv_tiles = {}

            def load_head(h):
                if h in qkv_tiles:
                    return qkv_tiles[h]
                qT_sb = qkv.tile([DK, S], bf16, tag="qT", name="qT_sb")
                nc.sync.dma_start(out=qT_sb[:], in_=qT_d[h])
                kT_sb = qkv.tile([DK, S], bf16, tag="kT", name="kT_sb")
                nc.sync.dma_start(out=kT_sb[:], in_=kT_d[h])
                v_sb = qkv.tile([128, NKC, DK + 1], bf16, tag="v", name="v_sb")
                nc.sync.dma_start(
                    out=v_sb[:], in_=v_d[h].rearrange("(c p) d -> p c d", p=128)
                )
                qkv_tiles[h] = (qT_sb, kT_sb, v_sb)
                return qkv_tiles[h]

            load_head(0)

            # resident mask tiles, one per q-block
            mts = []
            for qb in range(NQB):
                mt = maskp.tile([QB, S], bf16, tag=f"m{qb}", name=f"mask{qb}")
                nc.sync.dma_start(out=mt[:], in_=m_d[qb * QB:(qb + 1) * QB, :])
                mts.append(mt)

            def get_mask(qb):
                return mts[qb]

            def emit_B_chunk(st, i4):
                for c16 in range(i4 * 4, i4 * 4 + 4):
                    off = c16 * 128
                    tpt = tp_ps.tile([128, KSB], f32, tag="tpf",
                                     name="tpt", bufs=3)
                    for q4 in range(4):
                        nc.tensor.matmul(
                            tpt[:, q4 * 128:(q4 + 1) * 128],
                            st["pu"][q4][:, off:off + 128],
                            id_bf[:],
                            start=True, stop=True,
                        )
                    pt_t = pt_p.tile([128, KSB], bf16, tag="pT")
                    if c16 % 8 < 5:
                        nc.scalar.copy(pt_t[:], tpt[:])
                    else:
                        nc.vector.tensor_copy(pt_t[:], tpt[:])
                    st["pT"].append(pt_t)

            def emit_B_tail(st):
                h, qs, last = st["h"], st["qs"], st["last"]
                pu, pT, v_sb = st["pu"], st["pT"], st["v_sb"]
                xT = tp_ps.tile([128, KSB], f32, tag="tp", bufs=1)
                for kc in range(NKC):
                    nc.tensor.matmul(
                        xT[0:DK + 1, :],
                        v_sb[:, kc, :],
                        pT[kc][:],
                        start=(kc == 0), stop=(kc == NKC - 1),
                    )
                xT_sb = small.tile([DK + 1, KSB], f32, tag="xT", bufs=2)
                nc.scalar.copy(xT_sb[:], xT[0:DK + 1, :])

                xn = tp_ps.tile([128, 4 * (DK + 1)], f32, tag="tp", bufs=1)
                for q4 in range(4):
                    nc.tensor.transpose(
                        xn[:, q4 * (DK + 1):(q4 + 1) * (DK + 1)],
                        xT_sb[:, q4 * 128:(q4 + 1) * 128],
                        id_f32[0:DK + 1, 0:DK + 1],
                    )
                for q4 in range(4):
                    qb = qs * 4 + q4
                    rc = small.tile([QB, 1], f32, tag="rc")
                    nc.vector.reciprocal(rc[:], xn[:, q4 * (DK + 1) + DK:
                                                   q4 * (DK + 1) + DK + 1])
                    x_sb = xo_p.tile([QB, DK], f32, tag="x")
                    nc.vector.tensor_scalar(
                        out=x_sb[:],
                        in0=xn[:, q4 * (DK + 1):q4 * (DK + 1) + DK],
                        scalar1=rc[:], scalar2=None, op0=Alu.mult,
                    )
                    nc.sync.dma_start(
                        out=x_d[h, qb * QB:(qb + 1) * QB, :], in_=x_sb[:]
                    )
                    if not last:
                        pf = pf_p.tile([QB, S], f32, tag="pf")
                        nc.vector.tensor_scalar(
                            out=pf[:], in0=pu[q4][:],
                            scalar1=rc[:], scalar2=None, op0=Alu.mult,
                        )
                        nc.sync.dma_start(
                            out=p_d[h, qb * QB:(qb + 1) * QB, :],
                            in_=pf[:]
                        )

            for h in range(HPC):
                qT_sb, kT_sb, v_sb = load_head(h)
                if h + 1 < HPC:
                    load_head(h + 1)  # prefetch next head's inputs

                for qs in range(NSUP):
                    last = (h == HPC - 1 and qs == NSUP - 1)
                    pu = [pu_p.tile([QB, S], bf16, tag="pu", name="pu") for _ in range(4)]
                    st = {"h": h, "qs": qs, "last": last, "pu": pu,
                          "pT": [], "v_sb": v_sb}

                    for q4 in range(4):
                        qb = qs * 4 + q4
                        u_t = u_p.tile([QB, S], bf16, tag="u", name="u_t")
                        for k2 in range(S // EXPW):
                            s_t = s_ps.tile([QB, EXPW], f32, tag="s",
                                            name="s_t")
                            for j in range(EXPW // KSB):
                                nc.tensor.matmul(
                                    s_t[:, j * KSB:(j + 1) * KSB],
                                    qT_sb[:, qb * QB:(qb + 1) * QB],
                                    kT_sb[:, k2 * EXPW + j * KSB:
                                          k2 * EXPW + (j + 1) * KSB],
                                    start=True, stop=True,
                                )
                            nc.scalar.activation(
                                u_t[:, k2 * EXPW:(k2 + 1) * EXPW], s_t[:],
                                Act.Exp)
                        if not last:
                            nc.vector.tensor_mul(pu[q4][:], u_t[:],
                                                 get_mask(qb)[:])
                        else:
                            # final super: fused mask-mul + row-sum so the
                            # p normalize/DMA overlaps the PV chain instead
                            # of trailing it (shorter kernel tail)
                            r_e = small.tile([QB, 1], f32, tag="re",
                                             name="r_e")
                            nc.vector.scalar_tensor_tensor(
                                out=pu[q4][:], in0=u_t[:], scalar=1.0,
                                in1=get_mask(qb)[:], op0=Alu.mult,
                                op1=Alu.mult, accum_out=r_e[:])
                            rc_e = small.tile([QB, 1], f32, tag="rce",
                                              name="rc_e")
                            nc.vector.reciprocal(rc_e[:], r_e[:])
                            pf_e = pf_p.tile([QB, S], f32, tag="pf",
                                             name="pf_e")
                            nc.vector.tensor_scalar(
                                out=pf_e[:], in0=pu[q4][:], scalar1=rc_e[:],
                                scalar2=None, op0=Alu.mult)
                            nc.sync.dma_start(
                                out=p_d[h, qb * QB:(qb + 1) * QB, :],
                                in_=pf_e[:])
                    for i4 in range(4):
                        emit_B_chunk(st, i4)
                    emit_B_tail(st)

    nc.compile()
    return nc


def _get_nc():
    global _nc_cache
    if _nc_cache is None:
        _nc_cache = _build_nc()
    return _nc_cache


def _prep_in_maps(query, key, value, mask):
    bf = ml_dtypes.bfloat16
    q = (query.reshape(B * H, S, DK).transpose(0, 2, 1) * SCALE).astype(bf)
    kt = key.reshape(B * H, S, DK).transpose(0, 2, 1).astype(bf)
    v = np.ones((B * H, S, DK + 1), dtype=bf)
    v[:, :, :DK] = value.reshape(B * H, S, DK).astype(bf)
    mb = mask.reshape(B, S, S).astype(bf)
    in_maps = []
    for c in range(N_CORES):
        h0 = c * HPC
        in_maps.append({
            "qT": np.ascontiguousarray(q[h0:h0 + HPC]),
            "kT": np.ascontiguousarray(kt[h0:h0 + HPC]),
            "v": np.ascontiguousarray(v[h0:h0 + HPC]),
            "maskb": np.ascontiguousarray(mb[h0 // H]),
        })
    return in_maps


def _gather(results):
    p = np.concatenate([results[c]["p_out"] for c in range(N_CORES)], axis=0)
    x = np.concatenate([results[c]["x_out"] for c in range(N_CORES)], axis=0)
    return (x.reshape(B, H, S, DK), p.reshape(B, H, S, S))


def run(query, key, value, mask, trace=False):
    from concourse.bass_utils import run_bass_kernel_spmd

    nc = _get_nc()
    in_maps = _prep_in_maps(query, key, value, mask)
    res = None
    last_err = None
    for attempt in range(3):
        try:
            res = run_bass_kernel_spmd(
                nc, in_maps, core_ids=list(range(N_CORES)), trace=trace
            )
            break
        except Exception as e:  # transient device wedge: retry
            last_err = e
            import time
            time.sleep(5)
    if res is None:
        raise last_err
    x, p = _gather(res.results)
    return x, p, res


def kernel(query, key, value, mask):
    x, p, _ = run(query, key, value, mask)
    return (x, p)
